# revision 4
# baseline (speedup 1.0000x reference)
"""DeflateVertexToHex Trainium2 kernel.

out[b, t, :] = (mean over valid s of vertex_feats[b, hex_to_vertex[t, s], :]) @ W + b

Shapes: vertex_feats [4, 20000, 512] f32, hex_to_vertex [10000, 6] i64,
W [512, 512] f32, b [512] f32 -> out [4, 10000, 512] f32.

Sharding over 8 NeuronCores: core c handles batch c//2 and hex half c%2
(5000 hexes, padded to 5120 = 40 tiles of 128).

Per 128-hex tile on device:
  1. one SWDGE dma_gather pulls 768 rows (128 hexes x 6 slots, 2 KB each)
     from the batch vertex table in HBM into one [128, 3072] SBUF tile
     (partition = hex, free = slot-major). Padded (-1) adjacency slots are
     remapped host-side to an appended all-zero vertex row.
  2. DVE/ACT add-tree sums the 6 slots; one tensor_scalar multiplies by
     1/count (host-precomputed per hex).
  3. PE transposes pooled [hex, D] -> [D, hex] in 4 128x128 chunks (PSUM),
     DVE copies back to SBUF.
  4. PE matmul accumulates out[hex, H] = pooled.T^T @ W over 4 K-chunks.
  5. DVE adds the (host-broadcast) bias while copying PSUM -> SBUF,
     HWDGE writes the 128x512 output block to DRAM.
"""

import numpy as np

import concourse.bacc as bacc
import concourse.tile as tile
from concourse import mybir
from concourse.bass_utils import run_bass_kernel_spmd
from concourse.masks import make_identity

F32 = mybir.dt.float32
I16 = mybir.dt.int16

B = 4
N = 20000
D = 512
H = 512
T = 10000
S = 6
P = 128
N_CORES = 8
T_CORE = T // 2          # 5000 hexes per core
TILES = (T_CORE + P - 1) // P  # 40
PADT = TILES * P         # 5120
NIDX = S * P             # 768 gathers per tile
IDXW = NIDX // 16        # 48 idx columns per tile (16-partition wrap)


def _patch_tile_drain():
    """This container's walrus rejects >1 sync wait on the tail InstDrain
    emitted by TileContext; split the waits across single-wait SP nops."""
    if getattr(tile.TileContext, "_drain_patch_applied", False):
        return

    def _drain_and_barrier_split(self, tick_clock, wait_clock):
        nc = self.nc
        probe = nc.sync.nop(nofuse=True)
        wait_clock.add_sem_waits(
            probe.ins, tile.ScopedClock({None: tick_clock.global_clock})
        )
        si = probe.ins.sync_info
        waits = list(si.on_wait) if si is not None else []
        if si is not None:
            si.on_wait = []
        for w in waits[1:]:
            n = nc.sync.nop(nofuse=True)
            n.ins.sync_info = mybir.SyncInfo(on_wait=[w], on_update=[])
        if waits:
            probe.ins.sync_info = mybir.SyncInfo(on_wait=[waits[0]], on_update=[])
        nc.sync.drain()
        nc.all_engine_barrier()
        assert self.sems is not None
        popped = nc._tile_sem_poison_stack.pop()
        assert popped is self._sem_poison
        nc.clear_and_free_semaphores(list(self.sems.allocated().values()))
        nc.all_engine_barrier()

    tile.TileContext._drain_and_barrier = _drain_and_barrier_split
    tile.TileContext._drain_patch_applied = True


def build_module():
    _patch_tile_drain()
    nc = bacc.Bacc("TRN2", target_bir_lowering=False, debug=False)
    vtx = nc.declare_dram_parameter("vtx", [N + 1, D], F32, isOutput=False)
    wm = nc.declare_dram_parameter("wmat", [D, H], F32, isOutput=False)
    bbc = nc.declare_dram_parameter("bbc", [P, H], F32, isOutput=False)
    idx = nc.declare_dram_parameter("idx", [P, TILES * IDXW], I16, isOutput=False)
    invc = nc.declare_dram_parameter("invc", [P, TILES], F32, isOutput=False)
    out = nc.declare_dram_parameter("out", [PADT, H], F32, isOutput=True)

    with tile.TileContext(nc) as tc:
        with (
            tc.tile_pool(name="const", bufs=1) as constp,
            tc.tile_pool(name="gather", bufs=8) as gpool,
            tc.tile_pool(name="tmp", bufs=2) as tmp,
            tc.tile_pool(name="ptsb", bufs=2) as ptsb,
            tc.tile_pool(name="osb", bufs=3) as osb,
            tc.tile_pool(name="ptps", bufs=2, space="PSUM") as ptps,
            tc.tile_pool(name="mmps", bufs=2, space="PSUM") as mmps,
        ):
            w_sb = constp.tile([P, 4 * H], F32)
            for c in range(4):
                nc.sync.dma_start(
                    w_sb[:, c * H : (c + 1) * H], wm[c * P : (c + 1) * P, :]
                )
            b_sb = constp.tile([P, H], F32)
            nc.sync.dma_start(b_sb[:], bbc[:])
            idx_sb = constp.tile([P, TILES * IDXW], I16)
            nc.sync.dma_start(idx_sb[:], idx[:])
            invc_sb = constp.tile([P, TILES], F32)
            nc.sync.dma_start(invc_sb[:], invc[:])
            ident = constp.tile([P, P], F32)
            make_identity(nc, ident[:])

            for t in range(TILES):
                g = gpool.tile([P, S * D], F32)
                nc.gpsimd.dma_gather(
                    g[:].rearrange("p (s e) -> p s e", e=D),
                    vtx[:],
                    idx_sb[:, t * IDXW : (t + 1) * IDXW],
                    NIDX,
                    NIDX,
                    D,
                )
                s01 = tmp.tile([P, D], F32, tag="s01")
                nc.any.tensor_add(s01[:], g[:, 0:D], g[:, D : 2 * D])
                s23 = tmp.tile([P, D], F32, tag="s23")
                nc.any.tensor_add(s23[:], g[:, 2 * D : 3 * D], g[:, 3 * D : 4 * D])
                s45 = tmp.tile([P, D], F32, tag="s45")
                nc.any.tensor_add(s45[:], g[:, 4 * D : 5 * D], g[:, 5 * D : 6 * D])
                s0123 = tmp.tile([P, D], F32, tag="s0123")
                nc.any.tensor_add(s0123[:], s01[:], s23[:])
                sum6 = tmp.tile([P, D], F32, tag="sum6")
                nc.any.tensor_add(sum6[:], s0123[:], s45[:])
                pooled = tmp.tile([P, D], F32, tag="pooled")
                nc.vector.tensor_scalar_mul(pooled[:], sum6[:], invc_sb[:, t : t + 1])

                ptp = ptps.tile([P, D], F32)
                for c in range(4):
                    nc.tensor.transpose(
                        ptp[:, c * P : (c + 1) * P],
                        pooled[:, c * P : (c + 1) * P],
                        ident[:],
                    )
                ptile = ptsb.tile([P, D], F32)
                nc.vector.tensor_copy(ptile[:], ptp[:])

                mmp = mmps.tile([P, H], F32)
                for c in range(4):
                    nc.tensor.matmul(
                        mmp[:],
                        lhsT=ptile[:, c * P : (c + 1) * P],
                        rhs=w_sb[:, c * H : (c + 1) * H],
                        start=(c == 0),
                        stop=(c == 3),
                    )
                o = osb.tile([P, H], F32)
                nc.any.tensor_add(o[:], mmp[:], b_sb[:])
                nc.sync.dma_start(out[t * P : (t + 1) * P, :], o[:])
    # The PJRT exec path serializes nc as-is; finalize here so Bacc.compile
    # (register allocation, GPSIMD library-load insertion) has run.
    nc.finalize()
    return nc


def prep_inputs(vertex_feats, hex_to_vertex, W, b):
    """Host-side prep -> per-core in_maps."""
    vertex_feats = np.ascontiguousarray(np.asarray(vertex_feats, dtype=np.float32))
    hex_to_vertex = np.asarray(hex_to_vertex)
    W = np.ascontiguousarray(np.asarray(W, dtype=np.float32))
    b = np.asarray(b, dtype=np.float32)

    mask = hex_to_vertex >= 0
    safe = np.where(mask, hex_to_vertex, N).astype(np.int16)  # [T, 6]
    count = np.maximum(mask.sum(axis=1), 1).astype(np.float32)  # [T]
    inv = (1.0 / count).astype(np.float32)

    vtx_pads = []
    for bi in range(B):
        vp = np.zeros((N + 1, D), dtype=np.float32)
        vp[:N] = vertex_feats[bi]
        vtx_pads.append(vp)

    bbc = np.ascontiguousarray(np.broadcast_to(b, (P, H))).astype(np.float32)

    half_arrays = []
    for h in range(2):
        sl = slice(h * T_CORE, (h + 1) * T_CORE)
        safe_pad = np.full((PADT, S), N, dtype=np.int16)
        safe_pad[:T_CORE] = safe[sl]
        inv_pad = np.ones(PADT, dtype=np.float32)
        inv_pad[:T_CORE] = inv[sl]
        # flat[t, s*128 + p] = safe_pad[t*128 + p, s]
        flat = safe_pad.reshape(TILES, P, S).transpose(0, 2, 1).reshape(TILES, NIDX)
        # SWDGE idx wrap: [16, TILES*IDXW], column t*IDXW+j, row p = flat[t, j*16+p]
        idx16 = flat.reshape(TILES, IDXW, 16).transpose(2, 0, 1).reshape(16, TILES * IDXW)
        idx_full = np.tile(idx16, (8, 1))  # replicate across 8 Q7 core groups
        invc_arr = np.ascontiguousarray(inv_pad.reshape(TILES, P).T)  # [P, TILES]
        half_arrays.append((np.ascontiguousarray(idx_full), invc_arr))

    in_maps = []
    for c in range(N_CORES):
        bi, h = c // 2, c % 2
        idx_full, invc_arr = half_arrays[h]
        in_maps.append(
            {
                "vtx": vtx_pads[bi],
                "wmat": W,
                "bbc": bbc,
                "idx": idx_full,
                "invc": invc_arr,
            }
        )
    return in_maps


def assemble_output(results):
    out = np.empty((B, T, H), dtype=np.float32)
    for c in range(N_CORES):
        bi, h = c // 2, c % 2
        out[bi, h * T_CORE : (h + 1) * T_CORE] = results[c]["out"][:T_CORE]
    return out


_CACHE = {}


def kernel(vertex_feats, hex_to_vertex, W, b):
    nc = _CACHE.get("nc")
    if nc is None:
        nc = build_module()
        _CACHE["nc"] = nc
    in_maps = prep_inputs(vertex_feats, hex_to_vertex, W, b)
    res = run_bass_kernel_spmd(nc, in_maps, list(range(N_CORES)))
    return assemble_output(res.results)


if __name__ == "__main__":
    rng = np.random.default_rng(0)
    vf = rng.standard_normal((B, N, D), dtype=np.float32)
    h2v = rng.integers(-1, N, size=(T, S), dtype=np.int64)
    W = (rng.standard_normal((D, H)) / np.sqrt(D)).astype(np.float32)
    b = np.zeros(H, dtype=np.float32)
    out = kernel(vertex_feats=vf, hex_to_vertex=h2v, W=W, b=b)
    print("out", out.shape, out.dtype, float(np.abs(out).max()))


# revision 19
# speedup vs baseline: 1.0197x; 1.0197x over previous
"""DeflateVertexToHex Trainium2 kernel.

out[b, t, :] = (mean over valid s of vertex_feats[b, hex_to_vertex[t, s], :]) @ W + b

Shapes: vertex_feats [4, 20000, 512] f32, hex_to_vertex [10000, 6] i64,
W [512, 512] f32, b [512] f32 -> out [4, 10000, 512] f32.

Sharding over 8 NeuronCores: core c handles batch c//2 and hex half c%2
(5000 hexes, padded to 5120 = 40 tiles of 128).

Per 128-hex tile on device:
  1. one SWDGE dma_gather pulls 768 rows (128 hexes x 6 slots, 2 KB each)
     from the batch vertex table in HBM into one [128, 3072] SBUF tile
     (partition = hex, free = slot-major). Padded (-1) adjacency slots are
     remapped host-side to an appended all-zero vertex row.
  2. DVE/ACT add-tree sums the 6 slots; one tensor_scalar multiplies by
     1/count (host-precomputed per hex).
  3. PE transposes pooled [hex, D] -> [D, hex] in 4 128x128 chunks (PSUM),
     DVE copies back to SBUF.
  4. PE matmul accumulates out[hex, H] = pooled.T^T @ W over 4 K-chunks.
  5. DVE adds the (host-broadcast) bias while copying PSUM -> SBUF,
     HWDGE writes the 128x512 output block to DRAM.
"""

import numpy as np

import concourse.bacc as bacc
import concourse.tile as tile
from concourse import mybir
from concourse.bass_utils import run_bass_kernel_spmd
from concourse.masks import make_identity

F32 = mybir.dt.float32
I16 = mybir.dt.int16

B = 4
N = 20000
D = 512
H = 512
T = 10000
S = 6
P = 128
N_CORES = 8
T_CORE = T // 2          # 5000 hexes per core
TILES = (T_CORE + P - 1) // P  # 40
PADT = TILES * P         # 5120
NIDX = S * P             # 768 gathers per tile
IDXW = NIDX // 16        # 48 idx columns per tile (16-partition wrap)
LT_H = T_CORE - (TILES - 1) * P  # 8 real hexes in the last tile
LT_IDX = S * LT_H        # 48 gathers in the last tile (s-major, s*LT_H+h)


def _patch_tile_drain():
    """This container's walrus rejects >1 sync wait on the tail InstDrain
    emitted by TileContext; split the waits across single-wait SP nops."""
    if getattr(tile.TileContext, "_drain_patch_applied", False):
        return

    def _drain_and_barrier_split(self, tick_clock, wait_clock):
        nc = self.nc
        probe = nc.sync.nop(nofuse=True)
        wait_clock.add_sem_waits(
            probe.ins, tile.ScopedClock({None: tick_clock.global_clock})
        )
        si = probe.ins.sync_info
        waits = list(si.on_wait) if si is not None else []
        if si is not None:
            si.on_wait = []
        for w in waits[1:]:
            n = nc.sync.nop(nofuse=True)
            n.ins.sync_info = mybir.SyncInfo(on_wait=[w], on_update=[])
        if waits:
            probe.ins.sync_info = mybir.SyncInfo(on_wait=[waits[0]], on_update=[])
        nc.sync.drain()
        nc.all_engine_barrier()
        assert self.sems is not None
        popped = nc._tile_sem_poison_stack.pop()
        assert popped is self._sem_poison
        nc.clear_and_free_semaphores(list(self.sems.allocated().values()))
        nc.all_engine_barrier()

    tile.TileContext._drain_and_barrier = _drain_and_barrier_split
    tile.TileContext._drain_patch_applied = True


# Pooling strategy: how the 6 gathered slots are summed.
#   "dve_tree": 5 DVE adds, then 4 plain PE transposes of the sum
#   "pairs":    3 DVE pair-adds, then 12 PE transpose-accumulates
#   "pe_all":   24 PE transpose-accumulates, DVE idle
POOL_MODE = "pairs"
# Main matmul input dtype: float32 (exact) or float32r (TF32-class, ~1.4e-4
# rel err measured on HW, 4x faster on PE).
MM_F32R = False


GATHER_BUFS = 10
TMP_BUFS = 3
PT_BUFS = 2
OSB_BUFS = 4
PTPS_BUFS = 3
MMPS_BUFS = 3


GATHER_SPLIT = 3  # 1 = one 768-row gather per tile, 3 = three 256-row gathers


def build_module(pool_mode=None, mm_f32r=None, bufs=None, gather_split=None):
    pool_mode = POOL_MODE if pool_mode is None else pool_mode
    mm_f32r = MM_F32R if mm_f32r is None else mm_f32r
    gather_split = GATHER_SPLIT if gather_split is None else gather_split
    bufs = bufs or {}
    gb = bufs.get("g", GATHER_BUFS)
    tb = bufs.get("t", TMP_BUFS)
    pb = bufs.get("p", PT_BUFS)
    ob = bufs.get("o", OSB_BUFS)
    qb = bufs.get("q", PTPS_BUFS)
    mb = bufs.get("m", MMPS_BUFS)
    mm_dt = mybir.dt.float32r if mm_f32r else F32

    _patch_tile_drain()
    nc = bacc.Bacc("TRN2", target_bir_lowering=False, debug=False)
    vtx = nc.declare_dram_parameter("vtx", [N + 1, D], F32, isOutput=False)
    wm = nc.declare_dram_parameter("wmat", [D, H], mm_dt, isOutput=False)
    bbc = nc.declare_dram_parameter("bbc", [P, H], F32, isOutput=False)
    idx = nc.declare_dram_parameter("idx", [P, TILES * IDXW], I16, isOutput=False)
    invc = nc.declare_dram_parameter("invc", [P, TILES], F32, isOutput=False)
    # last-tile slot-pool selection matrix: sel[k, h] = (k < LT_IDX and
    # k % LT_H == h), zero rows beyond LT_IDX null out stale gather data
    sel = nc.declare_dram_parameter("sel", [P, LT_H], F32, isOutput=False)
    out = nc.declare_dram_parameter("out", [PADT, H], F32, isOutput=True)

    with tile.TileContext(nc) as tc:
        with (
            tc.tile_pool(name="const", bufs=1) as constp,
            tc.tile_pool(name="gather", bufs=gb) as gpool,
            tc.tile_pool(name="tmp", bufs=tb) as tmp,
            tc.tile_pool(name="ptsb", bufs=pb) as ptsb,
            tc.tile_pool(name="osb", bufs=ob) as osb,
            tc.tile_pool(name="ptps", bufs=qb, space="PSUM") as ptps,
            tc.tile_pool(name="mmps", bufs=mb, space="PSUM") as mmps,
        ):
            # idx first: it gates the first gather; W/bias/identity only gate
            # compute several microseconds later. Tile 0's slice goes in a
            # separate small DMA so gather 0 isn't held by the full load.
            idx_sb = constp.tile([P, TILES * IDXW], I16)
            nc.sync.dma_start(idx_sb[:, :IDXW], idx[:, :IDXW])
            nc.sync.dma_start(idx_sb[:, IDXW:], idx[:, IDXW:])
            w_sb = constp.tile([P, 4 * H], mm_dt)
            for c in range(4):
                nc.sync.dma_start(
                    w_sb[:, c * H : (c + 1) * H], wm[c * P : (c + 1) * P, :]
                )
            b_sb = constp.tile([P, H], F32)
            nc.sync.dma_start(b_sb[:], bbc[:])
            invc_sb = constp.tile([P, TILES], F32)
            nc.sync.dma_start(invc_sb[:], invc[:])
            sel_sb = constp.tile([P, LT_H], F32)
            nc.sync.dma_start(sel_sb[:], sel[:])
            ident = constp.tile([P, P], F32)
            make_identity(nc, ident[:])

            for t in range(TILES):
                if t == TILES - 1:
                    # 8 real hexes left: gather just their 48 rows (s-major,
                    # i = s*8+h -> partition i), then pool across partitions
                    # with a selection matmul: ptp[:, c*128 : c*128+8] =
                    # glast[:, chunk].T @ sel.
                    glast = gpool.tile([P, D], F32, tag="g0")
                    nc.gpsimd.dma_gather(
                        glast[:].rearrange("p (s e) -> p s e", e=D),
                        vtx[:],
                        idx_sb[:, t * IDXW : t * IDXW + LT_IDX // 16],
                        LT_IDX,
                        LT_IDX,
                        D,
                    )
                    ptp = ptps.tile([P, D], F32)
                    for c in range(4):
                        nc.tensor.matmul(
                            ptp[:, c * P : c * P + LT_H],
                            lhsT=glast[:LT_IDX, c * P : (c + 1) * P],
                            rhs=sel_sb[:LT_IDX, :],
                        )
                    ptile = ptsb.tile([P, D], mm_dt)
                    nc.scalar.copy(
                        ptile[:].rearrange("p (c e) -> p c e", c=4)[:, :, :LT_H],
                        ptp[:].rearrange("p (c e) -> p c e", c=4)[:, :, :LT_H],
                    )
                    mmp = mmps.tile([P, H], F32)
                    for c in range(4):
                        nc.tensor.matmul(
                            mmp[:LT_H, :],
                            lhsT=ptile[:, c * P : c * P + LT_H],
                            rhs=w_sb[:, c * H : (c + 1) * H],
                            start=(c == 0),
                            stop=(c == 3),
                        )
                    o = osb.tile([P, H], F32)
                    nc.vector.scalar_tensor_tensor(
                        o[:LT_H, :],
                        mmp[:LT_H, :],
                        invc_sb[:LT_H, t : t + 1],
                        b_sb[:LT_H, :],
                        op0=mybir.AluOpType.mult,
                        op1=mybir.AluOpType.add,
                    )
                    nc.sync.dma_start(out[t * P : t * P + LT_H, :], o[:LT_H, :])
                    continue
                if gather_split == 1:
                    g = gpool.tile([P, S * D], F32)
                    nc.gpsimd.dma_gather(
                        g[:].rearrange("p (s e) -> p s e", e=D),
                        vtx[:],
                        idx_sb[:, t * IDXW : (t + 1) * IDXW],
                        NIDX,
                        NIDX,
                        D,
                    )
                    gparts = [g[:, 2 * pi * D : (2 * pi + 2) * D] for pi in range(3)]
                else:
                    assert gather_split == 3
                    gparts = []
                    for pi in range(3):
                        gp = gpool.tile([P, 2 * D], F32, tag=f"g{pi}")
                        sub = NIDX // 3  # 256 indices = 2 slots
                        nc.gpsimd.dma_gather(
                            gp[:].rearrange("p (s e) -> p s e", e=D),
                            vtx[:],
                            idx_sb[:, t * IDXW + pi * 16 : t * IDXW + (pi + 1) * 16],
                            sub,
                            sub,
                            D,
                        )
                        gparts.append(gp)

                # slot sum, transposed into PSUM: ptp[:, c*128:(c+1)*128] =
                # (sum over slots)^T chunk c
                ptp = ptps.tile([P, D], F32)
                if pool_mode == "dve_tree":
                    s01 = tmp.tile([P, D], F32, tag="s01")
                    nc.any.tensor_add(s01[:], gparts[0][:, 0:D], gparts[0][:, D : 2 * D])
                    s23 = tmp.tile([P, D], F32, tag="s23")
                    nc.any.tensor_add(s23[:], gparts[1][:, 0:D], gparts[1][:, D : 2 * D])
                    s45 = tmp.tile([P, D], F32, tag="s45")
                    nc.any.tensor_add(s45[:], gparts[2][:, 0:D], gparts[2][:, D : 2 * D])
                    s0123 = tmp.tile([P, D], F32, tag="s0123")
                    nc.any.tensor_add(s0123[:], s01[:], s23[:])
                    sum6 = tmp.tile([P, D], F32, tag="sum6")
                    nc.any.tensor_add(sum6[:], s0123[:], s45[:])
                    parts = [sum6]
                elif pool_mode == "pairs":
                    parts = []
                    for pi in range(3):
                        sp = tmp.tile([P, D], F32, tag=f"pair{pi}")
                        nc.vector.tensor_add(
                            sp[:],
                            gparts[pi][:, 0:D],
                            gparts[pi][:, D : 2 * D],
                        )
                        parts.append(sp)
                elif pool_mode == "pe_all":
                    parts = [gp[:, s * D : (s + 1) * D] for gp in gparts for s in range(2)]
                else:
                    raise ValueError(pool_mode)

                for c in range(4):
                    for pi, sp in enumerate(parts):
                        src = sp[:, c * P : (c + 1) * P] if pool_mode != "pe_all" else sp[:, c * P : (c + 1) * P]
                        nc.tensor.matmul(
                            ptp[:, c * P : (c + 1) * P],
                            lhsT=src,
                            rhs=ident[:],
                            is_transpose=True,
                            start=(pi == 0),
                            stop=(pi == len(parts) - 1),
                        )

                ptile = ptsb.tile([P, D], mm_dt)
                nc.scalar.copy(ptile[:], ptp[:])

                mmp = mmps.tile([P, H], F32)
                for c in range(4):
                    nc.tensor.matmul(
                        mmp[:],
                        lhsT=ptile[:, c * P : (c + 1) * P],
                        rhs=w_sb[:, c * H : (c + 1) * H],
                        start=(c == 0),
                        stop=(c == 3),
                    )
                # out = (sumT^T @ W) * (1/count) + bias, fused on DVE
                o = osb.tile([P, H], F32)
                nc.vector.scalar_tensor_tensor(
                    o[:],
                    mmp[:],
                    invc_sb[:, t : t + 1],
                    b_sb[:],
                    op0=mybir.AluOpType.mult,
                    op1=mybir.AluOpType.add,
                )
                rows = P if t < TILES - 1 else T_CORE - (TILES - 1) * P
                nc.sync.dma_start(out[t * P : t * P + rows, :], o[:rows, :])
    # The PJRT exec path serializes nc as-is; finalize here so Bacc.compile
    # (register allocation, GPSIMD library-load insertion) has run.
    nc.finalize()
    return nc


def prep_inputs(vertex_feats, hex_to_vertex, W, b):
    """Host-side prep -> per-core in_maps."""
    vertex_feats = np.ascontiguousarray(np.asarray(vertex_feats, dtype=np.float32))
    hex_to_vertex = np.asarray(hex_to_vertex)
    W = np.ascontiguousarray(np.asarray(W, dtype=np.float32))
    b = np.asarray(b, dtype=np.float32)

    mask = hex_to_vertex >= 0
    safe = np.where(mask, hex_to_vertex, N).astype(np.int16)  # [T, 6]
    count = np.maximum(mask.sum(axis=1), 1).astype(np.float32)  # [T]
    inv = (1.0 / count).astype(np.float32)

    vtx_pads = []
    for bi in range(B):
        vp = np.zeros((N + 1, D), dtype=np.float32)
        vp[:N] = vertex_feats[bi]
        vtx_pads.append(vp)

    bbc = np.ascontiguousarray(np.broadcast_to(b, (P, H))).astype(np.float32)

    nfull = TILES - 1
    half_arrays = []
    for h in range(2):
        sl = slice(h * T_CORE, (h + 1) * T_CORE)
        safe_pad = np.full((PADT, S), N, dtype=np.int16)
        safe_pad[:T_CORE] = safe[sl]
        inv_pad = np.ones(PADT, dtype=np.float32)
        inv_pad[:T_CORE] = inv[sl]
        # full tiles: flat[t, s*128 + p] = safe_pad[t*128 + p, s]
        flat = (
            safe_pad[: nfull * P]
            .reshape(nfull, P, S)
            .transpose(0, 2, 1)
            .reshape(nfull, NIDX)
        )
        # SWDGE idx wrap: column t*IDXW+j, row p16 = flat[t, j*16+p16]
        idx16 = np.zeros((16, TILES * IDXW), dtype=np.int16)
        idx16[:, : nfull * IDXW] = (
            flat.reshape(nfull, IDXW, 16).transpose(2, 0, 1).reshape(16, nfull * IDXW)
        )
        # last tile: 48 indices, i = s*LT_H + h
        flat_last = (
            safe_pad[nfull * P : nfull * P + LT_H].T.reshape(LT_IDX)
        )  # [s, h] -> s*LT_H+h
        idx16[:, nfull * IDXW : nfull * IDXW + LT_IDX // 16] = flat_last.reshape(
            LT_IDX // 16, 16
        ).T
        idx_full = np.tile(idx16, (8, 1))  # replicate across 8 Q7 core groups
        invc_arr = np.ascontiguousarray(inv_pad.reshape(TILES, P).T)  # [P, TILES]
        half_arrays.append((np.ascontiguousarray(idx_full), invc_arr))

    sel_arr = np.zeros((P, LT_H), dtype=np.float32)
    k = np.arange(LT_IDX)
    sel_arr[k, k % LT_H] = 1.0

    in_maps = []
    for c in range(N_CORES):
        bi, h = c // 2, c % 2
        idx_full, invc_arr = half_arrays[h]
        in_maps.append(
            {
                "vtx": vtx_pads[bi],
                "wmat": W,
                "bbc": bbc,
                "idx": idx_full,
                "invc": invc_arr,
                "sel": sel_arr,
            }
        )
    return in_maps


def assemble_output(results):
    out = np.empty((B, T, H), dtype=np.float32)
    for c in range(N_CORES):
        bi, h = c // 2, c % 2
        out[bi, h * T_CORE : (h + 1) * T_CORE] = results[c]["out"][:T_CORE]
    return out


_CACHE = {}


def kernel(vertex_feats, hex_to_vertex, W, b):
    nc = _CACHE.get("nc")
    if nc is None:
        nc = build_module()
        _CACHE["nc"] = nc
    in_maps = prep_inputs(vertex_feats, hex_to_vertex, W, b)
    res = run_bass_kernel_spmd(nc, in_maps, list(range(N_CORES)))
    return assemble_output(res.results)


if __name__ == "__main__":
    rng = np.random.default_rng(0)
    vf = rng.standard_normal((B, N, D), dtype=np.float32)
    h2v = rng.integers(-1, N, size=(T, S), dtype=np.int64)
    W = (rng.standard_normal((D, H)) / np.sqrt(D)).astype(np.float32)
    b = np.zeros(H, dtype=np.float32)
    out = kernel(vertex_feats=vf, hex_to_vertex=h2v, W=W, b=b)
    print("out", out.shape, out.dtype, float(np.abs(out).max()))


# revision 20
# speedup vs baseline: 346.9571x; 340.2464x over previous
"""DeflateVertexToHex Trainium2 kernel.

out[b, t, :] = (mean over valid s of vertex_feats[b, hex_to_vertex[t, s], :]) @ W + b

Shapes: vertex_feats [4, 20000, 512] f32, hex_to_vertex [10000, 6] i64,
W [512, 512] f32, b [512] f32 -> out [4, 10000, 512] f32.

Sharding over 8 NeuronCores: core c handles batch c//2 and hex half c%2
(5000 hexes = 39 full 128-hex tiles + one 8-hex tail tile).

Per full 128-hex tile on device (defaults: POOL_MODE="pairs", GATHER_SPLIT=3):
  1. three SWDGE dma_gathers each pull 256 rows (128 hexes x 2 slots, 2 KB
     each) from the batch vertex table in HBM into [128, 1024] SBUF tiles
     (partition = hex). Padded (-1) adjacency slots are remapped host-side
     to an appended all-zero vertex row. Splitting the gather keeps the
     SDMA queue fine-grained so output writes interleave instead of
     queueing behind multi-MB gathers.
  2. DVE adds each gather pair -> three [128, 512] partial sums.
  3. PE transpose-accumulates the three partials into PSUM ([D, hex] layout,
     4 chunks x 3 partials = 12 128x128 transposes), ACT copies PSUM->SBUF.
  4. PE matmul accumulates out[hex, H] = sumT^T @ W over 4 K-chunks (fp32).
  5. one DVE scalar_tensor_tensor fuses the 1/count scale (moved across the
     matmul by linearity) and the bias: out = psum * invc + b; HWDGE writes
     the 128x512 block to DRAM.
The 8-hex tail tile gathers only its 48 real rows (s-major on partitions)
and pools them with a [48 x 8] selection matmul instead of transposes.
"""

import numpy as np

import concourse.bacc as bacc
import concourse.tile as tile
from concourse import mybir
from concourse.bass_utils import run_bass_kernel_spmd
from concourse.masks import make_identity

F32 = mybir.dt.float32
I16 = mybir.dt.int16

B = 4
N = 20000
D = 512
H = 512
T = 10000
S = 6
P = 128
N_CORES = 8
T_CORE = T // 2          # 5000 hexes per core
TILES = (T_CORE + P - 1) // P  # 40
PADT = TILES * P         # 5120
NIDX = S * P             # 768 gathers per tile
IDXW = NIDX // 16        # 48 idx columns per tile (16-partition wrap)
LT_H = T_CORE - (TILES - 1) * P  # 8 real hexes in the last tile
LT_IDX = S * LT_H        # 48 gathers in the last tile (s-major, s*LT_H+h)


def _patch_tile_drain():
    """This container's walrus rejects >1 sync wait on the tail InstDrain
    emitted by TileContext; split the waits across single-wait SP nops."""
    if getattr(tile.TileContext, "_drain_patch_applied", False):
        return

    def _drain_and_barrier_split(self, tick_clock, wait_clock):
        nc = self.nc
        probe = nc.sync.nop(nofuse=True)
        wait_clock.add_sem_waits(
            probe.ins, tile.ScopedClock({None: tick_clock.global_clock})
        )
        si = probe.ins.sync_info
        waits = list(si.on_wait) if si is not None else []
        if si is not None:
            si.on_wait = []
        for w in waits[1:]:
            n = nc.sync.nop(nofuse=True)
            n.ins.sync_info = mybir.SyncInfo(on_wait=[w], on_update=[])
        if waits:
            probe.ins.sync_info = mybir.SyncInfo(on_wait=[waits[0]], on_update=[])
        nc.sync.drain()
        nc.all_engine_barrier()
        assert self.sems is not None
        popped = nc._tile_sem_poison_stack.pop()
        assert popped is self._sem_poison
        nc.clear_and_free_semaphores(list(self.sems.allocated().values()))
        nc.all_engine_barrier()

    tile.TileContext._drain_and_barrier = _drain_and_barrier_split
    tile.TileContext._drain_patch_applied = True


# Pooling strategy: how the 6 gathered slots are summed.
#   "dve_tree": 5 DVE adds, then 4 plain PE transposes of the sum
#   "pairs":    3 DVE pair-adds, then 12 PE transpose-accumulates
#   "pe_all":   24 PE transpose-accumulates, DVE idle
POOL_MODE = "pairs"
# Main matmul input dtype: float32 (exact) or float32r (TF32-class, ~1.4e-4
# rel err measured on HW, 4x faster on PE).
MM_F32R = False


GATHER_BUFS = 10
TMP_BUFS = 3
PT_BUFS = 2
OSB_BUFS = 4
PTPS_BUFS = 3
MMPS_BUFS = 3


GATHER_SPLIT = 3  # 1 = one 768-row gather per tile, 3 = three 256-row gathers


def build_module(pool_mode=None, mm_f32r=None, bufs=None, gather_split=None):
    pool_mode = POOL_MODE if pool_mode is None else pool_mode
    mm_f32r = MM_F32R if mm_f32r is None else mm_f32r
    gather_split = GATHER_SPLIT if gather_split is None else gather_split
    bufs = bufs or {}
    gb = bufs.get("g", GATHER_BUFS)
    tb = bufs.get("t", TMP_BUFS)
    pb = bufs.get("p", PT_BUFS)
    ob = bufs.get("o", OSB_BUFS)
    qb = bufs.get("q", PTPS_BUFS)
    mb = bufs.get("m", MMPS_BUFS)
    mm_dt = mybir.dt.float32r if mm_f32r else F32

    _patch_tile_drain()
    nc = bacc.Bacc("TRN2", target_bir_lowering=False, debug=False)
    vtx = nc.declare_dram_parameter("vtx", [N + 1, D], F32, isOutput=False)
    wm = nc.declare_dram_parameter("wmat", [D, H], mm_dt, isOutput=False)
    bbc = nc.declare_dram_parameter("bbc", [P, H], F32, isOutput=False)
    idx = nc.declare_dram_parameter("idx", [P, TILES * IDXW], I16, isOutput=False)
    invc = nc.declare_dram_parameter("invc", [P, TILES], F32, isOutput=False)
    # last-tile slot-pool selection matrix: sel[k, h] = (k < LT_IDX and
    # k % LT_H == h), zero rows beyond LT_IDX null out stale gather data
    sel = nc.declare_dram_parameter("sel", [P, LT_H], F32, isOutput=False)
    out = nc.declare_dram_parameter("out", [PADT, H], F32, isOutput=True)

    with tile.TileContext(nc) as tc:
        with (
            tc.tile_pool(name="const", bufs=1) as constp,
            tc.tile_pool(name="gather", bufs=gb) as gpool,
            tc.tile_pool(name="tmp", bufs=tb) as tmp,
            tc.tile_pool(name="ptsb", bufs=pb) as ptsb,
            tc.tile_pool(name="osb", bufs=ob) as osb,
            tc.tile_pool(name="ptps", bufs=qb, space="PSUM") as ptps,
            tc.tile_pool(name="mmps", bufs=mb, space="PSUM") as mmps,
        ):
            # idx first: it gates the first gather; W/bias/identity only gate
            # compute several microseconds later. Tile 0's slice goes in a
            # separate small DMA so gather 0 isn't held by the full load.
            idx_sb = constp.tile([P, TILES * IDXW], I16)
            nc.sync.dma_start(idx_sb[:, :IDXW], idx[:, :IDXW])
            nc.sync.dma_start(idx_sb[:, IDXW:], idx[:, IDXW:])
            w_sb = constp.tile([P, 4 * H], mm_dt)
            for c in range(4):
                nc.sync.dma_start(
                    w_sb[:, c * H : (c + 1) * H], wm[c * P : (c + 1) * P, :]
                )
            b_sb = constp.tile([P, H], F32)
            nc.sync.dma_start(b_sb[:], bbc[:])
            invc_sb = constp.tile([P, TILES], F32)
            nc.sync.dma_start(invc_sb[:], invc[:])
            sel_sb = constp.tile([P, LT_H], F32)
            nc.sync.dma_start(sel_sb[:], sel[:])
            ident = constp.tile([P, P], F32)
            make_identity(nc, ident[:])

            for t in range(TILES):
                if t == TILES - 1:
                    # 8 real hexes left: gather just their 48 rows (s-major,
                    # i = s*8+h -> partition i), then pool across partitions
                    # with a selection matmul: ptp[:, c*128 : c*128+8] =
                    # glast[:, chunk].T @ sel.
                    glast = gpool.tile([P, D], F32, tag="g0")
                    nc.gpsimd.dma_gather(
                        glast[:].rearrange("p (s e) -> p s e", e=D),
                        vtx[:],
                        idx_sb[:, t * IDXW : t * IDXW + LT_IDX // 16],
                        LT_IDX,
                        LT_IDX,
                        D,
                    )
                    ptp = ptps.tile([P, D], F32)
                    for c in range(4):
                        nc.tensor.matmul(
                            ptp[:, c * P : c * P + LT_H],
                            lhsT=glast[:LT_IDX, c * P : (c + 1) * P],
                            rhs=sel_sb[:LT_IDX, :],
                        )
                    ptile = ptsb.tile([P, D], mm_dt)
                    nc.scalar.copy(
                        ptile[:].rearrange("p (c e) -> p c e", c=4)[:, :, :LT_H],
                        ptp[:].rearrange("p (c e) -> p c e", c=4)[:, :, :LT_H],
                    )
                    mmp = mmps.tile([P, H], F32)
                    for c in range(4):
                        nc.tensor.matmul(
                            mmp[:LT_H, :],
                            lhsT=ptile[:, c * P : c * P + LT_H],
                            rhs=w_sb[:, c * H : (c + 1) * H],
                            start=(c == 0),
                            stop=(c == 3),
                        )
                    o = osb.tile([P, H], F32)
                    nc.vector.scalar_tensor_tensor(
                        o[:LT_H, :],
                        mmp[:LT_H, :],
                        invc_sb[:LT_H, t : t + 1],
                        b_sb[:LT_H, :],
                        op0=mybir.AluOpType.mult,
                        op1=mybir.AluOpType.add,
                    )
                    nc.sync.dma_start(out[t * P : t * P + LT_H, :], o[:LT_H, :])
                    continue
                if gather_split == 1:
                    g = gpool.tile([P, S * D], F32)
                    nc.gpsimd.dma_gather(
                        g[:].rearrange("p (s e) -> p s e", e=D),
                        vtx[:],
                        idx_sb[:, t * IDXW : (t + 1) * IDXW],
                        NIDX,
                        NIDX,
                        D,
                    )
                    gparts = [g[:, 2 * pi * D : (2 * pi + 2) * D] for pi in range(3)]
                else:
                    assert gather_split == 3
                    gparts = []
                    for pi in range(3):
                        gp = gpool.tile([P, 2 * D], F32, tag=f"g{pi}")
                        sub = NIDX // 3  # 256 indices = 2 slots
                        nc.gpsimd.dma_gather(
                            gp[:].rearrange("p (s e) -> p s e", e=D),
                            vtx[:],
                            idx_sb[:, t * IDXW + pi * 16 : t * IDXW + (pi + 1) * 16],
                            sub,
                            sub,
                            D,
                        )
                        gparts.append(gp)

                # slot sum, transposed into PSUM: ptp[:, c*128:(c+1)*128] =
                # (sum over slots)^T chunk c
                ptp = ptps.tile([P, D], F32)
                if pool_mode == "dve_tree":
                    s01 = tmp.tile([P, D], F32, tag="s01")
                    nc.any.tensor_add(s01[:], gparts[0][:, 0:D], gparts[0][:, D : 2 * D])
                    s23 = tmp.tile([P, D], F32, tag="s23")
                    nc.any.tensor_add(s23[:], gparts[1][:, 0:D], gparts[1][:, D : 2 * D])
                    s45 = tmp.tile([P, D], F32, tag="s45")
                    nc.any.tensor_add(s45[:], gparts[2][:, 0:D], gparts[2][:, D : 2 * D])
                    s0123 = tmp.tile([P, D], F32, tag="s0123")
                    nc.any.tensor_add(s0123[:], s01[:], s23[:])
                    sum6 = tmp.tile([P, D], F32, tag="sum6")
                    nc.any.tensor_add(sum6[:], s0123[:], s45[:])
                    parts = [sum6]
                elif pool_mode == "pairs":
                    parts = []
                    for pi in range(3):
                        sp = tmp.tile([P, D], F32, tag=f"pair{pi}")
                        nc.vector.tensor_add(
                            sp[:],
                            gparts[pi][:, 0:D],
                            gparts[pi][:, D : 2 * D],
                        )
                        parts.append(sp)
                elif pool_mode == "pe_all":
                    parts = [gp[:, s * D : (s + 1) * D] for gp in gparts for s in range(2)]
                else:
                    raise ValueError(pool_mode)

                for c in range(4):
                    for pi, sp in enumerate(parts):
                        src = sp[:, c * P : (c + 1) * P] if pool_mode != "pe_all" else sp[:, c * P : (c + 1) * P]
                        nc.tensor.matmul(
                            ptp[:, c * P : (c + 1) * P],
                            lhsT=src,
                            rhs=ident[:],
                            is_transpose=True,
                            start=(pi == 0),
                            stop=(pi == len(parts) - 1),
                        )

                ptile = ptsb.tile([P, D], mm_dt)
                nc.scalar.copy(ptile[:], ptp[:])

                mmp = mmps.tile([P, H], F32)
                for c in range(4):
                    nc.tensor.matmul(
                        mmp[:],
                        lhsT=ptile[:, c * P : (c + 1) * P],
                        rhs=w_sb[:, c * H : (c + 1) * H],
                        start=(c == 0),
                        stop=(c == 3),
                    )
                # out = (sumT^T @ W) * (1/count) + bias, fused on DVE
                o = osb.tile([P, H], F32)
                nc.vector.scalar_tensor_tensor(
                    o[:],
                    mmp[:],
                    invc_sb[:, t : t + 1],
                    b_sb[:],
                    op0=mybir.AluOpType.mult,
                    op1=mybir.AluOpType.add,
                )
                rows = P if t < TILES - 1 else T_CORE - (TILES - 1) * P
                nc.sync.dma_start(out[t * P : t * P + rows, :], o[:rows, :])
    # The PJRT exec path serializes nc as-is; finalize here so Bacc.compile
    # (register allocation, GPSIMD library-load insertion) has run.
    nc.finalize()
    return nc


def prep_inputs(vertex_feats, hex_to_vertex, W, b):
    """Host-side prep -> per-core in_maps."""
    vertex_feats = np.ascontiguousarray(np.asarray(vertex_feats, dtype=np.float32))
    hex_to_vertex = np.asarray(hex_to_vertex)
    W = np.ascontiguousarray(np.asarray(W, dtype=np.float32))
    b = np.asarray(b, dtype=np.float32)

    mask = hex_to_vertex >= 0
    safe = np.where(mask, hex_to_vertex, N).astype(np.int16)  # [T, 6]
    count = np.maximum(mask.sum(axis=1), 1).astype(np.float32)  # [T]
    inv = (1.0 / count).astype(np.float32)

    vtx_pads = []
    for bi in range(B):
        vp = np.zeros((N + 1, D), dtype=np.float32)
        vp[:N] = vertex_feats[bi]
        vtx_pads.append(vp)

    bbc = np.ascontiguousarray(np.broadcast_to(b, (P, H))).astype(np.float32)

    nfull = TILES - 1
    half_arrays = []
    for h in range(2):
        sl = slice(h * T_CORE, (h + 1) * T_CORE)
        safe_pad = np.full((PADT, S), N, dtype=np.int16)
        safe_pad[:T_CORE] = safe[sl]
        inv_pad = np.ones(PADT, dtype=np.float32)
        inv_pad[:T_CORE] = inv[sl]
        # full tiles: flat[t, s*128 + p] = safe_pad[t*128 + p, s]
        flat = (
            safe_pad[: nfull * P]
            .reshape(nfull, P, S)
            .transpose(0, 2, 1)
            .reshape(nfull, NIDX)
        )
        # SWDGE idx wrap: column t*IDXW+j, row p16 = flat[t, j*16+p16]
        idx16 = np.zeros((16, TILES * IDXW), dtype=np.int16)
        idx16[:, : nfull * IDXW] = (
            flat.reshape(nfull, IDXW, 16).transpose(2, 0, 1).reshape(16, nfull * IDXW)
        )
        # last tile: 48 indices, i = s*LT_H + h
        flat_last = (
            safe_pad[nfull * P : nfull * P + LT_H].T.reshape(LT_IDX)
        )  # [s, h] -> s*LT_H+h
        idx16[:, nfull * IDXW : nfull * IDXW + LT_IDX // 16] = flat_last.reshape(
            LT_IDX // 16, 16
        ).T
        idx_full = np.tile(idx16, (8, 1))  # replicate across 8 Q7 core groups
        invc_arr = np.ascontiguousarray(inv_pad.reshape(TILES, P).T)  # [P, TILES]
        half_arrays.append((np.ascontiguousarray(idx_full), invc_arr))

    sel_arr = np.zeros((P, LT_H), dtype=np.float32)
    k = np.arange(LT_IDX)
    sel_arr[k, k % LT_H] = 1.0

    in_maps = []
    for c in range(N_CORES):
        bi, h = c // 2, c % 2
        idx_full, invc_arr = half_arrays[h]
        in_maps.append(
            {
                "vtx": vtx_pads[bi],
                "wmat": W,
                "bbc": bbc,
                "idx": idx_full,
                "invc": invc_arr,
                "sel": sel_arr,
            }
        )
    return in_maps


def assemble_output(results):
    out = np.empty((B, T, H), dtype=np.float32)
    for c in range(N_CORES):
        bi, h = c // 2, c % 2
        out[bi, h * T_CORE : (h + 1) * T_CORE] = results[c]["out"][:T_CORE]
    return out


_CACHE = {}


def kernel(vertex_feats, hex_to_vertex, W, b):
    nc = _CACHE.get("nc")
    if nc is None:
        nc = build_module()
        _CACHE["nc"] = nc
    in_maps = prep_inputs(vertex_feats, hex_to_vertex, W, b)
    res = run_bass_kernel_spmd(nc, in_maps, list(range(N_CORES)))
    return assemble_output(res.results)


if __name__ == "__main__":
    rng = np.random.default_rng(0)
    vf = rng.standard_normal((B, N, D), dtype=np.float32)
    h2v = rng.integers(-1, N, size=(T, S), dtype=np.int64)
    W = (rng.standard_normal((D, H)) / np.sqrt(D)).astype(np.float32)
    b = np.zeros(H, dtype=np.float32)
    out = kernel(vertex_feats=vf, hex_to_vertex=h2v, W=W, b=b)
    print("out", out.shape, out.dtype, float(np.abs(out).max()))
